# revision 1
# baseline (speedup 1.0000x reference)
"""Trainium2 Bass kernel for nn_ModelMamba (4-layer Mamba, B=8, L=2048).

Sharding: data-parallel over batch — 1 sequence per NeuronCore (8 cores).
Per-core layout: channels on partitions, time on the free dim.
Selective scan via the DVE TensorTensorScan instruction, one scan per
(d-chunk of 128 channels, state index n), time chunked in 2 super-chunks.

Self-contained: embedding + final head run host-side in numpy; the 4
Mamba layers (>99.9% of FLOPs) run on device.
"""
import os
import sys

for _p in ("/opt/trn_rl_repo", "/root/.axon_site/_ro/trn_rl_repo"):
    if os.path.isdir(_p) and _p not in sys.path:
        sys.path.append(_p)

import numpy as np
import ml_dtypes

import concourse.bacc as bacc
import concourse.bass as bass
import concourse.tile as tile
import concourse.mybir as mybir
from concourse.bass_utils import run_bass_kernel_spmd

F32 = mybir.dt.float32
BF16 = mybir.dt.bfloat16
ALU = mybir.AluOpType
AF = mybir.ActivationFunctionType
BF = ml_dtypes.bfloat16

# model dims (hardcoded per problem spec)
B, L = 8, 2048
D_MODEL, D_INNER, D_STATE, D_CONV, NL = 512, 1024, 16, 4, 4
DT_RANK = 32
E2 = 2 * D_INNER            # 2048 in_proj output channels
KC = D_MODEL // 128         # 4  k-chunks of d_model
DC = D_INNER // 128         # 8  d-chunks of d_inner
MC = E2 // 128              # 16 m-chunks of in_proj output
SC = 2                      # super-chunks over time
TCH = L // SC               # 1024
TQ = 512                    # matmul free-dim quantum
NQ = TCH // TQ              # 2 quarters per super-chunk

_CACHE = {}


def _build_nc():
    nc = bacc.Bacc(None, target_bir_lowering=False)

    # per-core input / output
    x0 = nc.dram_tensor("x0", [D_MODEL, L], BF16, kind="ExternalInput")
    x4 = nc.dram_tensor("x4", [D_MODEL, L], F32, kind="ExternalOutput")

    # weights (shared across cores), device layouts
    wiT_d = nc.dram_tensor("wiT", [NL, D_MODEL, E2], BF16, kind="ExternalInput")
    convd_d = nc.dram_tensor("convd", [NL, DC, D_CONV, 128, 128], BF16, kind="ExternalInput")
    wxT_d = nc.dram_tensor("wxT", [NL, D_INNER, 64], BF16, kind="ExternalInput")
    wdtT_d = nc.dram_tensor("wdtT", [NL, DT_RANK, D_INNER], BF16, kind="ExternalInput")
    woT_d = nc.dram_tensor("woT", [NL, D_INNER, D_MODEL], BF16, kind="ExternalInput")
    bdt_d = nc.dram_tensor("bdt", [NL, 128, DC], F32, kind="ExternalInput")
    cb_d = nc.dram_tensor("cb", [NL, 128, DC], F32, kind="ExternalInput")
    dp_d = nc.dram_tensor("dp", [NL, 128, DC], F32, kind="ExternalInput")
    acol_d = nc.dram_tensor("acol", [NL, 128, DC * D_STATE], F32, kind="ExternalInput")
    # DRAM staging for B/C row broadcast (per-core scratch)
    bcst_d = nc.dram_tensor("bcst", [NL * SC, 32, TCH], BF16, kind="Internal")

    with tile.TileContext(nc) as tc:
        with tc.tile_pool(name="wpool", bufs=1) as wp, \
             tc.tile_pool(name="planes", bufs=1) as pp, \
             tc.tile_pool(name="stream", bufs=1) as sp, \
             tc.tile_pool(name="psum", bufs=1, space="PSUM") as qp:

            # x tiles: one [128, TCH] tile per (super-chunk, k-chunk); reused in
            # place across layers (out_proj overwrites after in_proj reads).
            xt_pl = [[pp.tile([128, TCH], BF16, tag=f"x{s}_{k}", bufs=1,
                              name=f"x{s}_{k}") for k in range(KC)]
                     for s in range(SC)]
            for li in range(NL):
                # ---- load layer weights ----
                wiT = [wp.tile([128, E2], BF16, tag=f"wiT{k}", bufs=1,
                               name=f"wiT{k}") for k in range(KC)]
                for k in range(KC):
                    nc.sync.dma_start(out=wiT[k], in_=wiT_d[li, k * 128:(k + 1) * 128, :])
                convd = wp.tile([128, DC * D_CONV * 128], BF16, tag="convd", bufs=1,
                                name="convd")
                for d in range(DC):
                    for k in range(D_CONV):
                        nc.sync.dma_start(
                            out=convd[:, (d * D_CONV + k) * 128:(d * D_CONV + k + 1) * 128],
                            in_=convd_d[li, d, k])
                wxT = [wp.tile([128, 64], BF16, tag=f"wxT{k}", bufs=1,
                               name=f"wxT{k}") for k in range(DC)]
                for k in range(DC):
                    nc.sync.dma_start(out=wxT[k], in_=wxT_d[li, k * 128:(k + 1) * 128, :])
                wdtT = wp.tile([DT_RANK, D_INNER], BF16, tag="wdtT", bufs=1, name="wdtT")
                nc.sync.dma_start(out=wdtT, in_=wdtT_d[li])
                woT = [wp.tile([128, D_MODEL], BF16, tag=f"woT{k}", bufs=1,
                               name=f"woT{k}") for k in range(DC)]
                for k in range(DC):
                    nc.sync.dma_start(out=woT[k], in_=woT_d[li, k * 128:(k + 1) * 128, :])
                bdt_t = wp.tile([128, DC], F32, tag="bdt", bufs=1, name="bdt")
                nc.sync.dma_start(out=bdt_t, in_=bdt_d[li])
                cb_t = wp.tile([128, DC], F32, tag="cb", bufs=1, name="cb")
                nc.sync.dma_start(out=cb_t, in_=cb_d[li])
                dp_t = wp.tile([128, DC], F32, tag="dp", bufs=1, name="dp")
                nc.sync.dma_start(out=dp_t, in_=dp_d[li])
                acol_t = wp.tile([128, DC * D_STATE], F32, tag="acol", bufs=1, name="acol")
                nc.sync.dma_start(out=acol_t, in_=acol_d[li])

                if li == 0:
                    for s in range(SC):
                        for k in range(KC):
                            nc.sync.dma_start(
                                out=xt_pl[s][k],
                                in_=x0[k * 128:(k + 1) * 128, s * TCH:(s + 1) * TCH])

                # scan state + conv halo state across super-chunks
                htile = pp.tile([128, 128], F32, tag="hstate", bufs=1, name="hstate")
                tails = pp.tile([128, DC * 4], BF16, tag="ctail", bufs=1, name="ctail")

                for sc in range(SC):
                    t0 = sc * TCH
                    xin = xt_pl[sc]
                    # ---------- phase 1: in_proj ----------
                    xch = [sp.tile([128, TCH + 4], BF16, tag=f"xch{d}", bufs=1,
                                   name=f"xch{d}") for d in range(DC)]
                    szp = [sp.tile([128, TCH], BF16, tag=f"sz{d}", bufs=1,
                                   name=f"sz{d}") for d in range(DC)]
                    up = [sp.tile([128, TCH], BF16, tag=f"u{d}", bufs=1,
                                  name=f"u{d}") for d in range(DC)]
                    dtp = [sp.tile([128, TCH], BF16, tag=f"dt{d}", bufs=1,
                                   name=f"dt{d}") for d in range(DC)]
                    dtup = [sp.tile([128, TCH], BF16, tag=f"dtu{d}", bufs=1,
                                    name=f"dtu{d}") for d in range(DC)]
                    for d in range(DC):
                        if sc == 0:
                            nc.vector.memset(xch[d][:, 0:4], 0.0)
                        else:
                            nc.vector.tensor_copy(out=xch[d][:, 1:4],
                                                  in_=tails[:, d * 4:d * 4 + 3])
                    for q in range(NQ):
                        for m in range(MC):
                            ps = qp.tile([128, TQ], F32, tag="big", bufs=3, name="big")
                            for k in range(KC):
                                nc.tensor.matmul(
                                    ps, wiT[k][:, m * 128:(m + 1) * 128],
                                    xin[k][:, q * TQ:(q + 1) * TQ],
                                    start=(k == 0), stop=(k == KC - 1))
                            if m < DC:
                                nc.scalar.copy(
                                    out=xch[m][:, 4 + q * TQ: 4 + (q + 1) * TQ], in_=ps)
                            else:
                                nc.scalar.activation(
                                    out=szp[m - DC][:, q * TQ:(q + 1) * TQ], in_=ps,
                                    func=AF.Silu)
                    if sc + 1 < SC:
                        for d in range(DC):
                            nc.vector.tensor_copy(out=tails[:, d * 4:d * 4 + 3],
                                                  in_=xch[d][:, TCH + 1: TCH + 4])
                    # ---------- conv + silu -> u ----------
                    for d in range(DC):
                        for q in range(NQ):
                            ps = qp.tile([128, TQ], F32, tag="aux", bufs=3, name="aux")
                            for k in range(D_CONV):
                                nc.tensor.matmul(
                                    ps,
                                    convd[:, (d * D_CONV + k) * 128:(d * D_CONV + k + 1) * 128],
                                    xch[d][:, 1 + k + q * TQ: 1 + k + (q + 1) * TQ],
                                    start=(k == 0), stop=(k == D_CONV - 1))
                            nc.scalar.activation(
                                out=up[d][:, q * TQ:(q + 1) * TQ], in_=ps, func=AF.Silu,
                                bias=cb_t[:, d:d + 1])
                    # ---------- x_proj ----------
                    xdbl = sp.tile([64, TCH], BF16, tag="xdbl", bufs=2, name="xdbl")
                    for q in range(NQ):
                        ps = qp.tile([64, TQ], F32, tag="xp", bufs=2, name="xp")
                        for k in range(DC):
                            nc.tensor.matmul(ps, wxT[k],
                                             up[k][:, q * TQ:(q + 1) * TQ],
                                             start=(k == 0), stop=(k == DC - 1))
                        nc.scalar.copy(out=xdbl[:, q * TQ:(q + 1) * TQ], in_=ps)
                    # stage B/C rows to DRAM for partition-broadcast
                    nc.sync.dma_start(out=bcst_d[li * SC + sc], in_=xdbl[32:64, :])
                    # ---------- dt_proj + softplus = Ln(Exp(raw + bdt) + 1) ----------
                    for d in range(DC):
                        for q in range(NQ):
                            ps = qp.tile([128, TQ], F32, tag="aux", bufs=3, name="aux")
                            nc.tensor.matmul(ps, wdtT[:, d * 128:(d + 1) * 128],
                                             xdbl[0:DT_RANK, q * TQ:(q + 1) * TQ],
                                             start=True, stop=True)
                            esp = sp.tile([128, TQ], BF16, tag="esp", bufs=2, name="esp")
                            nc.scalar.activation(out=esp, in_=ps, func=AF.Exp,
                                                 bias=bdt_t[:, d:d + 1])
                            nc.scalar.activation(
                                out=dtp[d][:, q * TQ:(q + 1) * TQ], in_=esp,
                                func=AF.Ln, bias=1.0)
                        nc.vector.tensor_tensor(out=dtup[d], in0=dtp[d], in1=up[d],
                                                op=ALU.mult)
                    # ---------- phase 2: selective scan ----------
                    yac = [sp.tile([128, TCH], BF16, tag=f"yac{d}", bufs=1,
                                   name=f"yac{d}") for d in range(DC)]
                    for n in range(D_STATE):
                        bb = sp.tile([128, TCH], BF16, tag="bb", bufs=2, name="bb")
                        cc = sp.tile([128, TCH], BF16, tag="cc", bufs=2, name="cc")
                        src_b = bcst_d[li * SC + sc, n, :]
                        src_c = bcst_d[li * SC + sc, 16 + n, :]
                        nc.sync.dma_start(out=bb, in_=bass.AP(
                            tensor=src_b.tensor, offset=src_b.offset,
                            ap=[[0, 128]] + [list(x) for x in src_b.ap]))
                        nc.sync.dma_start(out=cc, in_=bass.AP(
                            tensor=src_c.tensor, offset=src_c.offset,
                            ap=[[0, 128]] + [list(x) for x in src_c.ap]))
                        for d in range(DC):
                            col = n * DC + d
                            da = sp.tile([128, TCH], BF16, tag="da", bufs=2, name="da")
                            nc.scalar.activation(
                                out=da, in_=dtp[d], func=AF.Exp,
                                scale=acol_t[:, d * D_STATE + n: d * D_STATE + n + 1])
                            xs = sp.tile([128, TCH], BF16, tag="xs", bufs=2, name="xs")
                            nc.vector.tensor_tensor(out=xs, in0=dtup[d], in1=bb,
                                                    op=ALU.mult)
                            h = sp.tile([128, TCH], BF16, tag="h", bufs=2, name="h")
                            nc.vector.tensor_tensor_scan(
                                out=h, data0=da, data1=xs,
                                initial=(0.0 if sc == 0 else htile[:, col:col + 1]),
                                op0=ALU.mult, op1=ALU.add)
                            if sc + 1 < SC:
                                nc.vector.tensor_copy(out=htile[:, col:col + 1],
                                                      in_=h[:, TCH - 1:TCH])
                            g = sp.tile([128, TCH], BF16, tag="g", bufs=2, name="g")
                            nc.vector.tensor_tensor(out=g, in0=h, in1=cc, op=ALU.mult)
                            if n == 0:
                                nc.vector.tensor_copy(out=yac[d], in_=g)
                            else:
                                nc.vector.tensor_tensor(out=yac[d], in0=yac[d], in1=g,
                                                        op=ALU.add)
                    # ---------- phase 3: gate + out_proj ----------
                    ygp = []
                    for d in range(DC):
                        y1 = sp.tile([128, TCH], BF16, tag="y1", bufs=2, name="y1")
                        nc.vector.scalar_tensor_tensor(
                            out=y1, in0=up[d], scalar=dp_t[:, d:d + 1], in1=yac[d],
                            op0=ALU.mult, op1=ALU.add)
                        yg = sp.tile([128, TCH], BF16, tag=f"yg{d}", bufs=1,
                                     name=f"yg{d}")
                        nc.vector.tensor_tensor(out=yg, in0=y1, in1=szp[d], op=ALU.mult)
                        ygp.append(yg)
                    for q in range(NQ):
                        for mo in range(KC):
                            ps = qp.tile([128, TQ], F32, tag="big", bufs=3, name="big")
                            for k in range(DC):
                                nc.tensor.matmul(
                                    ps, woT[k][:, mo * 128:(mo + 1) * 128],
                                    ygp[k][:, q * TQ:(q + 1) * TQ],
                                    start=(k == 0), stop=(k == DC - 1))
                            if li + 1 < NL:
                                nc.scalar.copy(
                                    out=xt_pl[sc][mo][:, q * TQ:(q + 1) * TQ], in_=ps)
                            else:
                                ostage = sp.tile([128, TQ], F32, tag="ost", bufs=2,
                                                 name="ost")
                                nc.scalar.copy(out=ostage, in_=ps)
                                nc.sync.dma_start(
                                    out=x4[mo * 128:(mo + 1) * 128,
                                           t0 + q * TQ: t0 + (q + 1) * TQ],
                                    in_=ostage)
    nc.finalize()
    return nc


def _prep_host(inputs):
    """Embedding + weight layout transforms (numpy)."""
    inp = {k: np.asarray(v) for k, v in inputs.items()}
    rna = inp["rna_data_pad"].astype(np.int64)
    tis = inp["tissue_id"].astype(np.int64)
    x0 = inp["seq_emb"][rna] + inp["tissue_emb"][tis][:, None, :]
    x0 = x0 * (rna != 0)[..., None].astype(np.float32)      # [B, L, D_MODEL]
    x0T = np.ascontiguousarray(x0.transpose(0, 2, 1)).astype(BF)  # [B, 512, L]

    wiT = np.ascontiguousarray(inp["W_in"].transpose(0, 2, 1)).astype(BF)       # [NL,512,2048]
    wxT = np.ascontiguousarray(inp["W_xproj"].transpose(0, 2, 1)).astype(BF)    # [NL,1024,64]
    wdtT = np.ascontiguousarray(inp["W_dt"].transpose(0, 2, 1)).astype(BF)      # [NL,32,1024]
    woT = np.ascontiguousarray(inp["W_out"].transpose(0, 2, 1)).astype(BF)      # [NL,1024,512]
    convd = np.zeros((NL, DC, D_CONV, 128, 128), np.float32)
    cw = inp["conv_w"]                                       # [NL, 1024, 4]
    idx = np.arange(128)
    for li in range(NL):
        for d in range(DC):
            for k in range(D_CONV):
                convd[li, d, k, idx, idx] = cw[li, d * 128:(d + 1) * 128, k]
    convd = convd.astype(BF)

    def cols(v):                                             # [NL,1024] -> [NL,128,DC]
        return np.ascontiguousarray(
            v.reshape(NL, DC, 128).transpose(0, 2, 1)).astype(np.float32)

    A = -np.exp(inp["A_log"].astype(np.float64)).astype(np.float32)  # [NL,1024,16]
    acol = np.ascontiguousarray(
        A.reshape(NL, DC, 128, D_STATE).transpose(0, 2, 1, 3).reshape(NL, 128, DC * D_STATE))

    w = {
        "wiT": wiT, "convd": convd, "wxT": wxT, "wdtT": wdtT, "woT": woT,
        "bdt": cols(inp["b_dt"]), "cb": cols(inp["conv_b"]), "dp": cols(inp["D_par"]),
        "acol": acol.astype(np.float32),
    }
    return inp, x0T, w


def _head(inp, x4_list):
    outs = []
    lens = inp["seq_lengths"].astype(np.int64)
    for b in range(B):
        x_last = x4_list[b][:, lens[b] - 1].astype(np.float32)
        h = np.maximum(x_last @ inp["W1"].T + inp["b1"], 0)
        outs.append(h @ inp["W2"].T + inp["b2"])
    return np.stack(outs).astype(np.float32)


def _run(inputs, trace=False):
    inp, x0T, w = _prep_host(inputs)
    if "nc" not in _CACHE:
        _CACHE["nc"] = _build_nc()
    nc = _CACHE["nc"]
    in_maps = [dict(w, x0=x0T[b]) for b in range(B)]
    kw = {}
    if trace:
        kw = dict(trace=True, trace_cores=[0])
    res = run_bass_kernel_spmd(nc, in_maps, core_ids=list(range(B)), **kw)
    out = _head(inp, [res.results[b]["x4"] for b in range(B)])
    return out, res


def kernel(**inputs) -> np.ndarray:
    out, _ = _run(inputs, trace=False)
    return out



# revision 9
# speedup vs baseline: 1.7340x; 1.7340x over previous
"""Trainium2 Bass kernel for nn_ModelMamba (4-layer Mamba, B=8, L=2048).

Sharding: tensor-parallel pairs. Cores (2w, 2w+1) form worker w; each
worker processes 2 sequences packed back-to-back on the time axis
(lengths padded to 16, ~1856 cols instead of 2048), and each core of the
pair owns half of d_inner (4 of 8 channel chunks) for the selective
scan / dt / gate / out_proj. The xc half of in_proj + conv + x_proj are
computed redundantly on both cores (PE has slack) so the only collective
is a 2-way AllReduce of out_proj partials per (layer, chunk).

All per-core variation lives in the DATA, not the program:
  - weights are permuted per core so local channels are chunks 0..3
  - sequence-boundary scan resets come from a per-core mask row that is
    added to dt after dtup is formed, driving dA -> 0 at boundaries.

Host side: embedding lookup, packing, weight slicing, final head.
"""
import math
import os
import sys

for _p in ("/opt/trn_rl_repo", "/root/.axon_site/_ro/trn_rl_repo"):
    if os.path.isdir(_p) and _p not in sys.path:
        sys.path.append(_p)

import numpy as np
import ml_dtypes

import concourse.bacc as bacc
import concourse.bass as bass
import concourse.tile as tile
import concourse.mybir as mybir
from concourse.bass_utils import run_bass_kernel_spmd

F32 = mybir.dt.float32
BF16 = mybir.dt.bfloat16
ALU = mybir.AluOpType
AF = mybir.ActivationFunctionType
BF = ml_dtypes.bfloat16

B, L = 8, 2048
D_MODEL, D_INNER, D_STATE, D_CONV, NL = 512, 1024, 16, 4, 4
DT_RANK = 32
KC = D_MODEL // 128          # 4 k-chunks of d_model
DCF = D_INNER // 128         # 8 full d_inner chunks (u)
DCL = 4                      # local d_inner chunks per core
MCX = 8                      # xc m-chunks (full)
MCZ = 4                      # local z m-chunks
MASK_HUGE = 30000.0
POOL_N0 = 4                  # n >= POOL_N0: g-mult + y-add run on gpsimd
RG = [[0, 1], [2, 3], [4, 5], [6, 7]]

_CACHE = {}


def _plan(lens):
    """Pack 8 sequences into 4 workers; return (T, chunks, pairs, offs)."""
    pl = [max(16, int(math.ceil((int(l) + 4) / 16.0) * 16)) for l in lens]
    order = sorted(range(B), key=lambda i: -pl[i])
    pairs = [(order[i], order[7 - i]) for i in range(4)]
    offs = {}
    tw = []
    for a, b in pairs:
        offs[a] = 0
        offs[b] = pl[a]
        tw.append(pl[a] + pl[b])
    T = max(tw)
    nch = max(1, int(math.ceil(T / 1024.0)))
    base = int(math.ceil(T / nch / 16.0) * 16)
    chunks = [base] * (nch - 1) + [T - base * (nch - 1)]
    assert chunks[-1] > 0 and sum(chunks) == T
    return T, tuple(chunks), pairs, offs


def _qsplit(cl):
    qs = [512] * (cl // 512)
    if cl % 512:
        qs.append(cl % 512)
    return qs


def _bcast_row(src):
    """DRAM row slice -> [128, n] broadcast AP."""
    return bass.AP(tensor=src.tensor, offset=src.offset,
                   ap=[[0, 128]] + [list(x) for x in src.ap])


def _build_nc(T, chunks):
    nch = len(chunks)
    nstg = NL * nch
    stages = [(li, ci) for li in range(NL) for ci in range(nch)]
    cstart = [sum(chunks[:i]) for i in range(nch)]

    nc = bacc.Bacc(None, target_bir_lowering=False, num_devices=8)

    x0 = nc.dram_tensor("x0", [D_MODEL, T], BF16, kind="ExternalInput")
    x4 = nc.dram_tensor("x4", [D_MODEL, T], BF16, kind="ExternalOutput")
    mask_d = nc.dram_tensor("maskrow", [1, T], BF16, kind="ExternalInput")

    wix_d = nc.dram_tensor("wix", [NL, D_MODEL, D_INNER], BF16, kind="ExternalInput")
    wiz_d = nc.dram_tensor("wiz", [NL, D_MODEL, 512], BF16, kind="ExternalInput")
    convd_d = nc.dram_tensor("convd", [NL, DCF, D_CONV, 128, 128], BF16,
                             kind="ExternalInput")
    wxT_d = nc.dram_tensor("wxT", [NL, D_INNER, 64], BF16, kind="ExternalInput")
    wdtT_d = nc.dram_tensor("wdtT", [NL, DT_RANK, 512], BF16, kind="ExternalInput")
    woT_d = nc.dram_tensor("woT", [NL, 512, D_MODEL], BF16, kind="ExternalInput")
    bdt_d = nc.dram_tensor("bdt", [NL, 128, DCL], F32, kind="ExternalInput")
    cb_d = nc.dram_tensor("cb", [NL, 128, DCF], F32, kind="ExternalInput")
    dp_d = nc.dram_tensor("dp", [NL, 128, DCL], F32, kind="ExternalInput")
    acol_d = nc.dram_tensor("acol", [NL, 128, DCL * D_STATE], F32,
                            kind="ExternalInput")
    bcst_d = nc.dram_tensor("bcst", [nstg, 32, 1024], BF16, kind="Internal")
    ccin_d = [nc.dram_tensor(f"ccin{s}", [D_MODEL, chunks[s % nch]], BF16)
              for s in range(nstg)]
    ccout_d = [nc.dram_tensor(f"ccout{s}", [D_MODEL, chunks[s % nch]], BF16)
               for s in range(nstg)]

    with tile.TileContext(nc) as tc:
        with tc.tile_pool(name="wpool", bufs=1) as wp, \
             tc.tile_pool(name="planes", bufs=1) as pp, \
             tc.tile_pool(name="stream", bufs=1) as sp, \
             tc.tile_pool(name="psum", bufs=1, space="PSUM") as qp:

            # persistent tiles -------------------------------------------------
            xt = [pp.tile([128, T], BF16, tag=f"xt{k}", bufs=1, name=f"xt{k}")
                  for k in range(KC)]
            htile = pp.tile([128, D_STATE * DCL], F32, tag="hstate", bufs=1,
                            name="hstate")

            # weight tile getters (tag ring, bufs=1 -> WAR-ordered reload)
            def w_wix():
                return [wp.tile([128, D_INNER], BF16, tag=f"wix{k}", bufs=1,
                                name=f"wix{k}") for k in range(KC)]

            def w_wiz():
                return [wp.tile([128, 512], BF16, tag=f"wiz{k}", bufs=1,
                                name=f"wiz{k}") for k in range(KC)]

            def w_convd():
                return wp.tile([128, DCF * D_CONV * 128], BF16, tag="convd",
                               bufs=1, name="convd")

            def w_wxT():
                return [wp.tile([128, 64], BF16, tag=f"wxT{k}", bufs=1,
                                name=f"wxT{k}") for k in range(DCF)]

            def w_wdtT():
                return wp.tile([DT_RANK, 512], BF16, tag="wdtT", bufs=1,
                               name="wdtT")

            def w_woT():
                # bufs=2: out_proj of layer li is emitted after layer li+1's
                # weight load; two live versions required.
                return [wp.tile([128, D_MODEL], BF16, tag=f"woT{k}", bufs=2,
                                name=f"woT{k}") for k in range(DCL)]

            def w_cols(tag):
                n = {"bdt": DCL, "cb": DCF, "dp": DCL,
                     "acol": DCL * D_STATE}[tag]
                bufs = 2 if tag in ("dp", "acol") else 1
                return wp.tile([128, n], F32, tag=tag, bufs=bufs, name=tag)

            # per-layer weight tile handles, filled by emit_weight_load
            W = {}

            def emit_weight_load(li):
                W["wix"] = w_wix()
                for k in range(KC):
                    nc.sync.dma_start(out=W["wix"][k],
                                      in_=wix_d[li, k * 128:(k + 1) * 128, :])
                W["wiz"] = w_wiz()
                for k in range(KC):
                    nc.sync.dma_start(out=W["wiz"][k],
                                      in_=wiz_d[li, k * 128:(k + 1) * 128, :])
                W["convd"] = w_convd()
                for d in range(DCF):
                    for k in range(D_CONV):
                        nc.sync.dma_start(
                            out=W["convd"][:, (d * D_CONV + k) * 128:
                                           (d * D_CONV + k + 1) * 128],
                            in_=convd_d[li, d, k])
                W["wxT"] = w_wxT()
                for k in range(DCF):
                    nc.sync.dma_start(out=W["wxT"][k],
                                      in_=wxT_d[li, k * 128:(k + 1) * 128, :])
                W["wdtT"] = w_wdtT()
                nc.sync.dma_start(out=W["wdtT"], in_=wdtT_d[li])
                W["woT"] = w_woT()
                for k in range(DCL):
                    nc.sync.dma_start(out=W["woT"][k],
                                      in_=woT_d[li, k * 128:(k + 1) * 128, :])
                for tag, src in (("bdt", bdt_d), ("cb", cb_d), ("dp", dp_d),
                                 ("acol", acol_d)):
                    W[tag] = w_cols(tag)
                    nc.sync.dma_start(out=W[tag], in_=src[li])

            # per-stage stream tiles (tag rings)
            def s_xch():
                return [sp.tile([128, 1024 + 4], BF16, tag=f"xch{d}", bufs=1,
                                name=f"xch{d}") for d in range(DCF)]

            def s_u():
                loc = [sp.tile([128, 1024], BF16, tag=f"ul{d}", bufs=2,
                               name=f"ul{d}") for d in range(DCL)]
                rem = [sp.tile([128, 1024], BF16, tag=f"ur{d}", bufs=1,
                               name=f"ur{d}") for d in range(DCL)]
                return loc + rem

            def s_szp():
                return [sp.tile([128, 1024], BF16, tag=f"sz{d}", bufs=2,
                                name=f"sz{d}") for d in range(MCZ)]

            def s_dt():
                return [sp.tile([128, 1024], BF16, tag=f"dt{d}", bufs=2,
                                name=f"dt{d}") for d in range(DCL)]

            def s_xdbl():
                return sp.tile([64, 1024], BF16, tag="xdbl", bufs=1,
                               name="xdbl")

            def s_maskt():
                return sp.tile([128, 1024], BF16, tag="maskt", bufs=2,
                               name="maskt")

            def s_yg():
                return [sp.tile([128, 1024], BF16, tag=f"yg{d}", bufs=2,
                                name=f"yg{d}") for d in range(DCL)]

            # stage state passed between emit phases
            ST = [dict() for _ in range(nstg)]

            def emit_front_portion(s, dl):
                li, ci = stages[s]
                cl = chunks[ci]
                qs = _qsplit(cl)
                st = ST[s]
                if dl == 0:
                    if ci == 0:
                        emit_weight_load(li)
                    st["W"] = dict(W)
                    if s == 0:
                        for k in range(KC):
                            nc.sync.dma_start(out=xt[k], in_=x0[k * 128:(k + 1) * 128, :])
                    st["xch"] = s_xch()
                    st["u"] = s_u()
                    st["szp"] = s_szp()
                    st["dt"] = s_dt()
                    st["yg"] = s_yg()
                if dl in (0, 1):
                    ms = range(0, 4) if dl == 0 else range(4, MCX)
                    qoff = 0
                    for qi, q in enumerate(qs):
                        for m in ms:
                            ps = qp.tile([128, 512], F32, tag="big", bufs=3,
                                         name="big")
                            for k in range(KC):
                                nc.tensor.matmul(
                                    ps[:, :q],
                                    st["W"]["wix"][k][:, m * 128:(m + 1) * 128],
                                    xt[k][:, cstart[ci] + qoff:
                                          cstart[ci] + qoff + q],
                                    start=(k == 0), stop=(k == KC - 1))
                            nc.scalar.copy(
                                out=st["xch"][m][:, 4 + qoff:4 + qoff + q],
                                in_=ps[:, :q])
                        qoff += q
                if dl == 1:
                    xch = st["xch"]
                    # conv halo: zeros at worker start, saved tails after
                    if ci == 0:
                        for d in range(DCF):
                            nc.vector.memset(xch[d][:, 0:4], 0.0)
                    else:
                        tl = ST[s - 1]["tails_tile"]
                        for d in range(DCF):
                            nc.vector.tensor_copy(out=xch[d][:, 1:4],
                                                  in_=tl[:, d * 4:d * 4 + 3])
                    if ci + 1 < nch:
                        tl = sp.tile([128, DCF * 4], BF16, tag="ctail", bufs=2,
                                     name="ctail")
                        for d in range(DCF):
                            nc.vector.tensor_copy(
                                out=tl[:, d * 4:d * 4 + 3],
                                in_=xch[d][:, cl + 1:cl + 4])
                        st["tails_tile"] = tl
                    # conv matmuls + silu block
                    for d in range(DCF):
                        qoff = 0
                        for q in qs:
                            ps = qp.tile([128, 512], F32, tag="aux", bufs=3,
                                         name="aux")
                            for k in range(D_CONV):
                                nc.tensor.matmul(
                                    ps[:, :q],
                                    st["W"]["convd"][:, (d * D_CONV + k) * 128:
                                               (d * D_CONV + k + 1) * 128],
                                    xch[d][:, 1 + k + qoff:1 + k + qoff + q],
                                    start=(k == 0), stop=(k == D_CONV - 1))
                            nc.scalar.activation(
                                out=st["u"][d][:, qoff:qoff + q], in_=ps[:, :q],
                                func=AF.Silu, bias=st["W"]["cb"][:, d:d + 1])
                            qoff += q
                if dl == 2:
                    # z (local) + silus, then xproj
                    qoff = 0
                    for q in qs:
                        for mz in range(MCZ):
                            ps = qp.tile([128, 512], F32, tag="big", bufs=3,
                                         name="big")
                            for k in range(KC):
                                nc.tensor.matmul(
                                    ps[:, :q],
                                    st["W"]["wiz"][k][:, mz * 128:(mz + 1) * 128],
                                    xt[k][:, cstart[ci] + qoff:
                                          cstart[ci] + qoff + q],
                                    start=(k == 0), stop=(k == KC - 1))
                            nc.scalar.activation(
                                out=st["szp"][mz][:, qoff:qoff + q],
                                in_=ps[:, :q], func=AF.Silu)
                        qoff += q
                    xdbl = s_xdbl()
                    st["xdbl"] = xdbl
                    qoff = 0
                    for q in qs:
                        ps = qp.tile([64, 512], F32, tag="xp", bufs=2,
                                     name="xp")
                        for k in range(DCF):
                            nc.tensor.matmul(ps[:, :q], st["W"]["wxT"][k],
                                             st["u"][k][:, qoff:qoff + q],
                                             start=(k == 0), stop=(k == DCF - 1))
                        nc.scalar.copy(out=xdbl[:, qoff:qoff + q],
                                       in_=ps[:, :q])
                        qoff += q
                if dl == 3:
                    xdbl = st["xdbl"]
                    nc.sync.dma_start(out=bcst_d[s, :, 0:cl],
                                      in_=xdbl[32:64, 0:cl])
                    # dt-proj + softplus (Exp then Ln, both in exp/ln table)
                    for d in range(DCL):
                        qoff = 0
                        for q in qs:
                            ps = qp.tile([128, 512], F32, tag="aux", bufs=3,
                                         name="aux")
                            nc.tensor.matmul(ps[:, :q],
                                             st["W"]["wdtT"][:, d * 128:(d + 1) * 128],
                                             xdbl[0:DT_RANK, qoff:qoff + q],
                                             start=True, stop=True)
                            esp = sp.tile([128, 512], BF16, tag="esp", bufs=1,
                                          name="esp")
                            nc.scalar.activation(out=esp[:, :q], in_=ps[:, :q],
                                                 func=AF.Exp,
                                                 bias=st["W"]["bdt"][:, d:d + 1])
                            nc.scalar.activation(
                                out=st["dt"][d][:, qoff:qoff + q],
                                in_=esp[:, :q], func=AF.Ln, bias=1.0)
                            qoff += q
                    mt = s_maskt()
                    st["maskt"] = mt
                    src = mask_d[0, cstart[ci]:cstart[ci] + cl]
                    nc.sync.dma_start(out=mt[:, 0:cl], in_=_bcast_row(src))
                    st["bb"] = {}
                    st["cc"] = {}
                    for n in range(2):
                        _prefetch_bc(s, n)

            def _prefetch_bc(s, n):
                li, ci = stages[s]
                cl = chunks[ci]
                st = ST[s]
                bb = sp.tile([128, 1024], BF16, tag="bb", bufs=3, name="bb")
                cc = sp.tile([128, 1024], BF16, tag="cc", bufs=3, name="cc")
                nc.sync.dma_start(out=bb[:, 0:cl],
                                  in_=_bcast_row(bcst_d[s, n, 0:cl]))
                nc.sync.dma_start(out=cc[:, 0:cl],
                                  in_=_bcast_row(bcst_d[s, 16 + n, 0:cl]))
                st["bb"][n] = bb
                st["cc"][n] = cc

            def emit_scan_prelude(s):
                """dtup per dl, then mask dt in place; allocate accumulators."""
                li, ci = stages[s]
                cl = chunks[ci]
                st = ST[s]
                st["dtup"] = [sp.tile([128, 1024], BF16, tag=f"dtup{d}",
                                      bufs=1, name=f"dtup{d}")
                              for d in range(DCL)]
                st["yacD"] = [sp.tile([128, 1024], BF16, tag=f"yacD{d}",
                                      bufs=1, name=f"yacD{d}")
                              for d in range(DCL)]
                st["yacP"] = [sp.tile([128, 1024], BF16, tag=f"yacP{d}",
                                      bufs=1, name=f"yacP{d}")
                              for d in range(DCL)]
                for dl in range(DCL):
                    dtt = st["dt"][dl]
                    nc.vector.tensor_tensor(out=st["dtup"][dl][:, :cl],
                                            in0=dtt[:, :cl],
                                            in1=st["u"][dl][:, :cl],
                                            op=ALU.mult)
                    nc.vector.tensor_tensor(out=dtt[:, :cl], in0=dtt[:, :cl],
                                            in1=st["maskt"][:, :cl],
                                            op=ALU.add)

            def emit_scan_n(s, n):
                """One state index: load-once bb/cc, run all 4 local d-chunks."""
                li, ci = stages[s]
                cl = chunks[ci]
                st = ST[s]
                if n + 2 < D_STATE and (n + 2) not in st["bb"]:
                    _prefetch_bc(s, n + 2)
                for dl in range(DCL):
                    dtt = st["dt"][dl]
                    da = sp.tile([128, 1024], BF16, tag="da", bufs=3,
                                 name="da")
                    col = dl * D_STATE + n
                    nc.scalar.activation(
                        out=da[:, :cl], in_=dtt[:, :cl], func=AF.Exp,
                        scale=st["W"]["acol"][:, col:col + 1])
                    xs = sp.tile([128, 1024], BF16, tag="xs", bufs=2,
                                 name="xs")
                    nc.vector.tensor_tensor(out=xs[:, :cl],
                                            in0=st["dtup"][dl][:, :cl],
                                            in1=st["bb"][n][:, :cl],
                                            op=ALU.mult)
                    h = sp.tile([128, 1024], BF16, tag="h", bufs=3, name="h")
                    hcol = n * DCL + dl
                    nc.vector.tensor_tensor_scan(
                        out=h[:, :cl], data0=da[:, :cl], data1=xs[:, :cl],
                        initial=(0.0 if ci == 0 else htile[:, hcol:hcol + 1]),
                        op0=ALU.mult, op1=ALU.add)
                    if ci + 1 < nch:
                        nc.scalar.copy(out=htile[:, hcol:hcol + 1],
                                       in_=h[:, cl - 1:cl])
                    ccn = st["cc"][n][:, :cl]
                    if n < POOL_N0:
                        if n == 0:
                            nc.vector.tensor_tensor(out=st["yacD"][dl][:, :cl],
                                                    in0=h[:, :cl], in1=ccn,
                                                    op=ALU.mult)
                        else:
                            g = sp.tile([128, 1024], BF16, tag="g", bufs=2,
                                        name="g")
                            nc.vector.tensor_tensor(out=g[:, :cl],
                                                    in0=h[:, :cl], in1=ccn,
                                                    op=ALU.mult)
                            nc.vector.tensor_tensor(out=st["yacD"][dl][:, :cl],
                                                    in0=st["yacD"][dl][:, :cl],
                                                    in1=g[:, :cl], op=ALU.add)
                    else:
                        if n == POOL_N0:
                            nc.gpsimd.tensor_tensor(out=st["yacP"][dl][:, :cl],
                                                    in0=h[:, :cl], in1=ccn,
                                                    op=ALU.mult)
                        else:
                            g = sp.tile([128, 1024], BF16, tag="gp", bufs=2,
                                        name="gp")
                            nc.gpsimd.tensor_tensor(out=g[:, :cl],
                                                    in0=h[:, :cl], in1=ccn,
                                                    op=ALU.mult)
                            nc.gpsimd.tensor_tensor(out=st["yacP"][dl][:, :cl],
                                                    in0=st["yacP"][dl][:, :cl],
                                                    in1=g[:, :cl], op=ALU.add)

            def emit_gate(s):
                li, ci = stages[s]
                cl = chunks[ci]
                st = ST[s]
                for dl in range(DCL):
                    y1 = sp.tile([128, 1024], BF16, tag="y1", bufs=1,
                                 name="y1")
                    nc.vector.scalar_tensor_tensor(
                        out=y1[:, :cl], in0=st["u"][dl][:, :cl],
                        scalar=st["W"]["dp"][:, dl:dl + 1],
                        in1=st["yacD"][dl][:, :cl],
                        op0=ALU.mult, op1=ALU.add)
                    nc.vector.tensor_tensor(out=y1[:, :cl], in0=y1[:, :cl],
                                            in1=st["yacP"][dl][:, :cl],
                                            op=ALU.add)
                    nc.vector.tensor_tensor(out=st["yg"][dl][:, :cl],
                                            in0=y1[:, :cl],
                                            in1=st["szp"][dl][:, :cl],
                                            op=ALU.mult)

            def emit_out(s):
                li, ci = stages[s]
                cl = chunks[ci]
                qs = _qsplit(cl)
                st = ST[s]
                qoff = 0
                for q in qs:
                    for mo in range(KC):
                        ps = qp.tile([128, 512], F32, tag="big", bufs=3,
                                     name="big")
                        for k in range(DCL):
                            nc.tensor.matmul(
                                ps[:, :q],
                                st["W"]["woT"][k][:, mo * 128:(mo + 1) * 128],
                                st["yg"][k][:, qoff:qoff + q],
                                start=(k == 0), stop=(k == DCL - 1))
                        ost = sp.tile([128, 512], BF16, tag="ost", bufs=2,
                                      name="ost")
                        nc.scalar.copy(out=ost[:, :q], in_=ps[:, :q])
                        if li + 1 < NL:
                            nc.sync.dma_start(
                                out=ccin_d[s][mo * 128:(mo + 1) * 128,
                                              qoff:qoff + q],
                                in_=ost[:, :q])
                        else:
                            nc.sync.dma_start(
                                out=x4[mo * 128:(mo + 1) * 128,
                                       cstart[ci] + qoff:cstart[ci] + qoff + q],
                                in_=ost[:, :q])
                    qoff += q
                if li + 1 < NL:
                    nc.gpsimd.collective_compute(
                        "AllReduce", ALU.add, replica_groups=RG,
                        ins=[ccin_d[s][:, :].opt()],
                        outs=[ccout_d[s][:, :].opt()])
                    for k in range(KC):
                        nc.sync.dma_start(
                            out=xt[k][:, cstart[ci]:cstart[ci] + cl],
                            in_=ccout_d[s][k * 128:(k + 1) * 128, :])

            # ---------------- schedule ----------------
            for dl in range(DCL):
                emit_front_portion(0, dl)
            for s in range(nstg):
                if s > 0:
                    emit_out(s - 1)
                emit_scan_prelude(s)
                for grp in range(4):
                    for n in range(4 * grp, 4 * grp + 4):
                        emit_scan_n(s, n)
                    if s + 1 < nstg:
                        emit_front_portion(s + 1, grp)
                emit_gate(s)
            emit_out(nstg - 1)
    nc.finalize()
    return nc


def _prep_host(inputs):
    inp = {k: np.asarray(v) for k, v in inputs.items()}
    rna = inp["rna_data_pad"].astype(np.int64)
    tis = inp["tissue_id"].astype(np.int64)
    lens = inp["seq_lengths"].astype(np.int64)
    T, chunks, pairs, offs = _plan(lens)

    x0 = inp["seq_emb"][rna] + inp["tissue_emb"][tis][:, None, :]
    x0 = x0 * (rna != 0)[..., None].astype(np.float32)
    x0T = np.ascontiguousarray(x0.transpose(0, 2, 1)).astype(np.float32)

    Wi = inp["W_in"].astype(np.float32)         # [NL, 2048, 512]
    cw = inp["conv_w"].astype(np.float32)       # [NL, 1024, 4]
    cbv = inp["conv_b"].astype(np.float32)      # [NL, 1024]
    Wx = inp["W_xproj"].astype(np.float32)      # [NL, 64, 1024]
    Wdt = inp["W_dt"].astype(np.float32)        # [NL, 1024, 32]
    bdtv = inp["b_dt"].astype(np.float32)
    A = -np.exp(inp["A_log"].astype(np.float64)).astype(np.float32)
    Dp = inp["D_par"].astype(np.float32)
    Wo = inp["W_out"].astype(np.float32)        # [NL, 512, 1024]

    idx = np.arange(128)

    def cols(v, ndc):                            # [NL, ndc*128] -> [NL,128,ndc]
        return np.ascontiguousarray(
            v.reshape(NL, ndc, 128).transpose(0, 2, 1)).astype(np.float32)

    in_maps = []
    for w in range(4):
        a, b = pairs[w]
        xp = np.zeros((D_MODEL, T), np.float32)
        xp[:, 0:lens[a]] = x0T[a][:, 0:lens[a]]
        xp[:, offs[b]:offs[b] + lens[b]] = x0T[b][:, 0:lens[b]]
        xp = xp.astype(BF)
        mrow = np.zeros((1, T), np.float32)
        mrow[0, offs[b]] = MASK_HUGE
        mrow = mrow.astype(BF)
        for h_ in range(2):
            lo = h_ * 512
            perm = np.concatenate([np.arange(lo, lo + 512),
                                   np.arange((1 - h_) * 512, (1 - h_) * 512 + 512)])
            wix = np.ascontiguousarray(
                Wi[:, perm, :].transpose(0, 2, 1)).astype(BF)    # [NL,512,1024]
            wiz = np.ascontiguousarray(
                Wi[:, D_INNER + lo:D_INNER + lo + 512, :]
                .transpose(0, 2, 1)).astype(BF)                  # [NL,512,512]
            convd = np.zeros((NL, DCF, D_CONV, 128, 128), np.float32)
            cwp = cw[:, perm, :]
            for li in range(NL):
                for d in range(DCF):
                    for k in range(D_CONV):
                        convd[li, d, k, idx, idx] = cwp[li, d * 128:(d + 1) * 128, k]
            wxT = np.ascontiguousarray(
                Wx[:, :, perm].transpose(0, 2, 1)).astype(BF)    # [NL,1024,64]
            wdtT = np.ascontiguousarray(
                Wdt[:, lo:lo + 512, :].transpose(0, 2, 1)).astype(BF)  # [NL,32,512]
            woT = np.ascontiguousarray(
                Wo[:, :, lo:lo + 512].transpose(0, 2, 1)).astype(BF)   # [NL,512,512]
            acl = A[:, lo:lo + 512, :]                           # [NL,512,16]
            acol = np.ascontiguousarray(
                acl.reshape(NL, DCL, 128, D_STATE).transpose(0, 2, 1, 3)
                .reshape(NL, 128, DCL * D_STATE))
            m = dict(
                x0=xp, maskrow=mrow, wix=wix, wiz=wiz,
                convd=convd.astype(BF), wxT=wxT, wdtT=wdtT, woT=woT,
                bdt=cols(bdtv[:, lo:lo + 512].reshape(NL, -1), DCL),
                cb=cols(cbv[:, perm].reshape(NL, -1), DCF),
                dp=cols(Dp[:, lo:lo + 512].reshape(NL, -1), DCL),
                acol=acol.astype(np.float32),
            )
            in_maps.append(m)
    return inp, lens, T, chunks, pairs, offs, in_maps


def _head(inp, lens, pairs, offs, results):
    xw = []
    for w in range(4):
        xw.append(results[2 * w]["x4"].astype(np.float32) +
                  results[2 * w + 1]["x4"].astype(np.float32))
    outs = np.zeros((B, 1), np.float32)
    W1, b1 = inp["W1"].astype(np.float32), inp["b1"].astype(np.float32)
    W2, b2 = inp["W2"].astype(np.float32), inp["b2"].astype(np.float32)
    for w in range(4):
        for sidx in pairs[w]:
            x_last = xw[w][:, offs[sidx] + lens[sidx] - 1]
            h = np.maximum(x_last @ W1.T + b1, 0)
            outs[sidx] = h @ W2.T + b2
    return outs


def _run(inputs, trace=False):
    inp, lens, T, chunks, pairs, offs, in_maps = _prep_host(inputs)
    key = (T, chunks)
    if key not in _CACHE:
        _CACHE[key] = _build_nc(T, chunks)
    nc = _CACHE[key]
    kw = {}
    if trace:
        kw = dict(trace=True, trace_cores=[0])
    res = run_bass_kernel_spmd(nc, in_maps, core_ids=list(range(8)), **kw)
    out = _head(inp, lens, pairs, offs, res.results)
    return out, res


def kernel(**inputs) -> np.ndarray:
    out, _ = _run(inputs, trace=False)
    return out
